# revision 76
# baseline (speedup 1.0000x reference)
"""FAVOR+ (Performer) linear attention on 8 Trainium2 NeuronCores.

Math (per batch b, head h, with m = hd = 64, scale = hd**-0.25):
  qkv = x @ W_qkv.T ; q,k,v : [N, H, hd]
  phi(z) = exp(scale*z @ rfs[h] - 0.5*|scale*z|^2)          (z = q or k)
  causal scan:  S_t = S_{t-1} + phi_k[t] (x) v[t] ; z_t = z_{t-1} + phi_k[t]
                out[t] = (phi_q[t] @ S_t) / (phi_q[t] . z_t + 1e-16)

Sharding: data-parallel over batch B=8, one batch per core.

Device-side algebra (everything bf16 into the PE, fp32 PSUM accum):
  - rfs[h] is orthogonal * sqrt(hd), so rfs@rfs^T = hd*I. Fold it into the
    projection on the host: proj = x @ (scale * W^T @ rfs), and recover the
    stabilizer |scale*z|^2 = |proj|^2 / hd.
  - The q-side stabilizer exp(-|z_q|^2/2) scales num and den identically and
    cancels in the ratio (den >> 1e-16 for this data), so phi_q = exp(proj_q).
  - k-side keeps it: phi_k = exp(proj_k - |proj_k|^2/(2*hd)), realized by a
    matmul with a constant -1/(2*hd) block-matrix on proj_k^2.

Per-core chunked scan (chunk L=128 tokens, v carries a ones column):
  AT   = phi_k_chunk @ phi_q_chunk^T            [j, i]  (PE, feature-major)
  ATm  = AT * triu_mask                                 (DVE, 4 heads/op)
  num' = phi_q @ [S | z]  +  ATm^T @ [v | 1]    [i, 65] (PE, PSUM-accum)
  S   += phi_k^T @ [v | 1]                              (PE)
  out  = num'[:, :64] / (num'[:, 64] + 1e-16)           (DVE recip, scalar mul)

Layout rule learned on HW: bf16 matmuls whose stationary operand alternates
SBUF base partitions (0 vs 64) crash the PE (concurrent row-tiles into one
PSUM bank).  Everything here keeps lhsT/rhs at base partition 0: odd heads'
phi halves are DMA-copied (partition shift) into their own [64, 512] tiles,
the scan state lives as [64, 780] (head-major columns), and the state PSUM
uses two banks at partitions 0-63.  phi_k token-major tiles come from the
DMA XBAR transpose (16-bit SBUF->SBUF), keeping the PE free for matmuls.
"""

import numpy as np

B, N, C, H = 8, 4096, 768, 12
HD = 64
G = H // 2            # head pairs stacked on 128 partitions
NCH = 512             # tokens per outer chunk
NSUB = NCH // 128     # 128-token scan chunks per outer chunk
NBIG = N // NCH
SCALE = HD ** -0.25

_CACHE = {}


def _build_bass():
    import concourse.bass as bass
    import concourse.mybir as mybir
    import concourse.tile as tile
    from concourse import bacc
    from contextlib import ExitStack

    f32 = mybir.dt.float32
    bf16 = mybir.dt.bfloat16
    AF = mybir.ActivationFunctionType

    nc = bacc.Bacc("TRN2", target_bir_lowering=False)
    xT = nc.declare_dram_parameter("xT", [C, N], bf16, isOutput=False)
    WT = nc.declare_dram_parameter("WT", [C, 3 * C], bf16, isOutput=False)
    nhalf = nc.declare_dram_parameter("nhalf", [128, 128], bf16, isOutput=False)
    mask4 = nc.declare_dram_parameter("mask4", [128, 512], f32, isOutput=False)
    outd = nc.declare_dram_parameter("out", [N, C], f32, isOutput=True)

    with tile.TileContext(nc) as tc, ExitStack() as ctx:
        consts = ctx.enter_context(tc.tile_pool(name="consts", bufs=1))
        xt_p = ctx.enter_context(tc.tile_pool(name="xt", bufs=2))
        phi_p = ctx.enter_context(tc.tile_pool(name="phi", bufs=2))
        odd_p = ctx.enter_context(tc.tile_pool(name="odd", bufs=2))
        sq_p = ctx.enter_context(tc.tile_pool(name="sq", bufs=2))
        tm_p = ctx.enter_context(tc.tile_pool(name="tm", bufs=2))
        v_p = ctx.enter_context(tc.tile_pool(name="v", bufs=2))
        atm_p = ctx.enter_context(tc.tile_pool(name="atm", bufs=3))
        st_p = ctx.enter_context(tc.tile_pool(name="st", bufs=2))
        stb_p = ctx.enter_context(tc.tile_pool(name="stb", bufs=2))
        den_p = ctx.enter_context(tc.tile_pool(name="den", bufs=4))
        out_p = ctx.enter_context(tc.tile_pool(name="outp", bufs=5))

        pp_ps = ctx.enter_context(tc.tile_pool(name="pp", bufs=4, space="PSUM"))
        num_ps = ctx.enter_context(tc.tile_pool(name="nm", bufs=2, space="PSUM"))
        at_ps = ctx.enter_context(tc.tile_pool(name="at", bufs=2, space="PSUM"))

        # ---- constants ----
        wt = []
        for ct in range(6):
            t = consts.tile([128, 3 * C], bf16, tag=f"wt{ct}")
            nc.gpsimd.dma_start(out=t[:], in_=WT[ct * 128:(ct + 1) * 128, :])
            wt.append(t)
        nh_sb = consts.tile([128, 128], bf16, tag="nh")
        nc.gpsimd.dma_start(out=nh_sb[:], in_=nhalf[:])
        mk_sb = consts.tile([128, 512], f32, tag="mk")
        nc.gpsimd.dma_start(out=mk_sb[:], in_=mask4[:])

        prev_f32 = None
        prev_bf = None
        pending_stores = []

        def flush_stores():
            while pending_stores:
                dst, src = pending_stores.pop(0)
                nc.sync.dma_start(out=dst, in_=src)

        def load_xt(cb):
            nsl = slice(cb * NCH, (cb + 1) * NCH)
            xt = []
            for ct in range(6):
                t = xt_p.tile([128, NCH], bf16, tag=f"xt{ct}", name="xt")
                nc.gpsimd.dma_start(out=t[:], in_=xT[ct * 128:(ct + 1) * 128, nsl])
                xt.append(t)
            return xt

        xt = load_xt(0)
        for cb in range(NBIG):
            n0 = cb * NCH

            # ---- v projection (token-major, ones column interleaved) ----
            vsb = []
            for nt in range(NSUB):
                vt = v_p.tile([128, H * 65], bf16, tag=f"v{nt}")
                ones = vt[:].rearrange("p (h d) -> p h d", d=65)[:, :, 64:65]
                nc.vector.memset(ones, 1.0)
                for half in range(2):
                    # v psum rides the num pool: its banks are idle during
                    # the proj phase, and this frees pp slots so k-tile
                    # projections never wait on trailing exps
                    pv = num_ps.tile([128, 512], f32, tag="num", name="pv")
                    fsl = slice(2 * C + half * 384, 2 * C + (half + 1) * 384)
                    for ct in range(6):
                        nc.tensor.matmul(
                            pv[:, :384], xt[ct][:, nt * 128:(nt + 1) * 128],
                            wt[ct][:, fsl],
                            start=(ct == 0), stop=(ct == 5))
                    dst = vt[:, half * 390:(half + 1) * 390].rearrange(
                        "p (h d) -> p h d", d=65)[:, :, :64]
                    src = pv[:, :384].rearrange("p (h d) -> p h d", d=64)
                    nc.scalar.copy(dst, src)
                vsb.append(vt)

            # prefetch next chunk's x^T now: ahead of the odd-head copies on
            # the gpsimd queue (those stall on exps), so the loads are in
            # flight a full phase before the next chunk's first matmuls
            xt_next = load_xt(cb + 1) if cb + 1 < NBIG else None

            # ---- q/k projection + feature maps (feature-major) ----
            # phi pair tiles hold heads (2g, 2g+1) on partitions 0-63/64-127;
            # odd heads get a DMA partition-shift into base-0 [64, 512] tiles.
            # k's stabilizer chain (sqr -> nhalf-mm -> exp) is deferred one
            # tile so the PE never waits on the scalar square.
            phiq, phik = [None] * H, [None] * H
            tmk = [[None] * NSUB for _ in range(G)]
            pair_k = [None] * G
            pending = []

            def finish_phi(isq, g, ph_):
                dst = phiq if isq == 0 else phik
                dst[2 * g] = ph_
                po = odd_p.tile([64, NCH], bf16, tag=f"po{isq}_{g}")
                nc.gpsimd.dma_start(out=po[:], in_=ph_[64:128, :])
                dst[2 * g + 1] = po
                if isq == 1:
                    pair_k[g] = ph_
                    # transposes ride sync+scalar rings EXCLUSIVELY (mixing
                    # DmaTransposeAnt with regular DMAs on one ring caused
                    # nondeterministic stale reads).  sync's (subs 0,2) go
                    # now; scalar's (subs 1,3) are deferred past the proj
                    # loop so they never delay the sqr/exp chain.
                    for sub in (0, 2):
                        t = tm_p.tile([128, 128], bf16, tag=f"tm{g}_{sub}",
                                      name="tm")
                        nc.sync.dma_start(
                            out=t[:], in_=ph_[:, sub * 128:(sub + 1) * 128],
                            transpose=True)
                        tmk[g][sub] = t

            def flush_pending():
                if not pending:
                    return
                pf_, sqr_, ph_, g_ = pending.pop(0)
                nc.tensor.matmul(pf_[:], nh_sb[:], sqr_[:],
                                 start=False, stop=True, skip_group_check=True)
                nc.scalar.activation(ph_[:], pf_[:], AF.Exp)
                finish_phi(1, g_, ph_)

            for isq in (1, 0):  # k tiles first: tmk transposes start early
                for g in range(G):
                    ft = isq * 6 + g
                    pf = pp_ps.tile([128, NCH], f32, tag="pp", name="pf")
                    for ct in range(6):
                        nc.tensor.matmul(
                            pf[:], wt[ct][:, ft * 128:(ft + 1) * 128], xt[ct][:],
                            start=(ct == 0), stop=(ct == 5))
                    ph = phi_p.tile([128, NCH], bf16, tag=f"ph{isq}_{g}")
                    if isq == 0:
                        flush_pending()
                        nc.scalar.activation(ph[:], pf[:], AF.Exp)
                        finish_phi(0, g, ph)
                    else:
                        flush_pending()
                        sqr = sq_p.tile([128, NCH], bf16, tag="sqr")
                        nc.scalar.square(sqr[:], pf[:])
                        pending.append((pf, sqr, ph, g))
            while pending:
                flush_pending()

            # deferred scalar-ring transposes (subs 1, 3)
            for sub in (1, 3):
                for g in range(G):
                    t = tm_p.tile([128, 128], bf16, tag=f"tm{g}_{sub}",
                                  name="tm")
                    nc.scalar.dma_start(
                        out=t[:], in_=pair_k[g][:, sub * 128:(sub + 1) * 128],
                        transpose=True)
                    tmk[g][sub] = t

            # previous chunk's stores: normalize long done, so these issue
            # instantly and can't delay the odd-head copies behind them
            flush_stores()

            def phs(lst, h, csl):
                """phi slice for head h over column slice csl, base part. 0."""
                t = lst[h]
                if h % 2 == 0:
                    return t[0:64, csl]
                return t[:, csl]

            # ---- causal scan over 128-token chunks ----
            for sub in range(NSUB):
                ci = cb * NSUB + sub
                ssl = slice(sub * 128, (sub + 1) * 128)
                first = (ci == 0)

                atms = [None] * 3

                def at_group(grp3):
                    pa = at_ps.tile([128, 512], f32, tag="at", name="pa")
                    for k_ in range(4):
                        h = grp3 * 4 + k_
                        nc.tensor.matmul(
                            pa[:, k_ * 128:(k_ + 1) * 128],
                            phs(phik, h, ssl), phs(phiq, h, ssl),
                            start=(k_ == 0), stop=(k_ == 3))
                    atm = atm_p.tile([128, 512], bf16, tag="atm", name="atm")
                    nc.vector.tensor_mul(atm[:], pa[:], mk_sb[:])
                    atms[grp3] = atm

                nums = [num_ps.tile([128, 512], f32, tag="num", name="pn")
                        for _ in range(2)]

                def num_bank(grp6):
                    pn = nums[grp6]
                    for hh in range(6):
                        h = grp6 * 6 + hh
                        opener = (hh == 0)
                        if not first:
                            nc.tensor.matmul(
                                pn[:, hh * 65:hh * 65 + 65],
                                phs(phiq, h, ssl),
                                prev_bf[:, h * 65:(h + 1) * 65],
                                start=opener, stop=False)
                            opener = False
                        nc.tensor.matmul(
                            pn[:, hh * 65:hh * 65 + 65],
                            atms[h // 4][:, (h % 4) * 128:(h % 4 + 1) * 128],
                            vsb[sub][:, h * 65:(h + 1) * 65],
                            start=opener, stop=(hh == 5))

                at_group(0)
                at_group(1)

                # state update: S += phi_k^T @ [v | 1], two banks at part. 0-63
                sts = [pp_ps.tile([128, 512], f32, tag="pp", name="pst")
                       for _ in range(2)]
                for h in range(H):
                    g, e = h // 2, h % 2
                    hh = h % 6
                    nc.tensor.matmul(
                        sts[h // 6][0:64, hh * 65:(hh + 1) * 65],
                        tmk[g][sub][:, e * 64:(e + 1) * 64],
                        vsb[sub][:, h * 65:(h + 1) * 65],
                        start=(hh == 0), stop=(hh == 5))

                num_bank(0)
                at_group(2)
                num_bank(1)

                new_f32 = st_p.tile([64, H * 65], f32, tag="stf")
                if first:
                    nc.vector.tensor_copy(new_f32[:, :390], sts[0][0:64, :390])
                    nc.vector.tensor_copy(new_f32[:, 390:], sts[1][0:64, :390])
                else:
                    nc.vector.tensor_add(new_f32[:, :390], sts[0][0:64, :390],
                                         prev_f32[:, :390])
                    nc.vector.tensor_add(new_f32[:, 390:], sts[1][0:64, :390],
                                         prev_f32[:, 390:])
                new_bf = stb_p.tile([64, H * 65], bf16, tag="stb")
                nc.vector.tensor_copy(new_bf[:], new_f32[:])
                prev_f32, prev_bf = new_f32, new_bf

                # ---- normalize and store ----
                den = den_p.tile([128, H], f32, tag="den")
                for grp6 in range(2):
                    src = nums[grp6][:, :390].rearrange(
                        "p (h d) -> p h d", d=65)[:, :, 64:65]
                    dst = den[:, grp6 * 6:(grp6 + 1) * 6].rearrange(
                        "p (h o) -> p h o", o=1)
                    nc.vector.tensor_copy(dst, src)
                rec = den_p.tile([128, H], f32, tag="rec")
                nc.vector.tensor_scalar_add(rec[:], den[:], 1e-16)
                nc.vector.reciprocal(rec[:], rec[:])
                ot = out_p.tile([128, C], f32, tag="out")
                for h in range(H):
                    nc.vector.tensor_scalar_mul(
                        ot[:, h * 64:(h + 1) * 64],
                        nums[h // 6][:, (h % 6) * 65:(h % 6) * 65 + 64],
                        rec[:, h:h + 1])
                pending_stores.append(
                    (outd[n0 + sub * 128:n0 + (sub + 1) * 128, :], ot[:]))

            xt = xt_next
        flush_stores()

    if not nc.is_finalized():
        nc.finalize()
    return nc


def _host_inputs(x, W_qkv, rfs):
    import ml_dtypes
    bf = ml_dtypes.bfloat16

    x = np.asarray(x, dtype=np.float32)
    W = np.asarray(W_qkv, dtype=np.float64)
    rfs = np.asarray(rfs, dtype=np.float64)

    Wq = W[:C].reshape(H, HD, C)
    Wk = W[C:2 * C].reshape(H, HD, C)
    Wv = W[2 * C:]
    cols = [SCALE * Wq[h].T @ rfs[h] for h in range(H)]
    cols += [SCALE * Wk[h].T @ rfs[h] for h in range(H)]
    cols.append(Wv.T)
    WT = np.ascontiguousarray(np.concatenate(cols, axis=1)).astype(bf)

    nhalf = np.zeros((128, 128), np.float32)
    nhalf[:64, :64] = -1.0 / (2 * HD)
    nhalf[64:, 64:] = -1.0 / (2 * HD)
    nhalf = nhalf.astype(bf)
    mask4 = np.ascontiguousarray(
        np.tile(np.triu(np.ones((128, 128), np.float32)), (1, 4)))

    shared = {"WT": WT, "nhalf": nhalf, "mask4": mask4}
    in_maps = []
    for b in range(B):
        m = {"xT": np.ascontiguousarray(x[b].T).astype(bf)}
        m.update(shared)
        in_maps.append(m)
    return in_maps


def kernel(x, W_qkv, rfs):
    from concourse.bass_utils import run_bass_kernel_spmd

    if "nc" not in _CACHE:
        _CACHE["nc"] = _build_bass()
    nc = _CACHE["nc"]
    in_maps = _host_inputs(x, W_qkv, rfs)
    res = run_bass_kernel_spmd(nc, in_maps, list(range(B)))
    return np.stack([res.results[b]["out"] for b in range(B)], axis=0)


# revision 78
# speedup vs baseline: 1.1399x; 1.1399x over previous
"""FAVOR+ (Performer) linear attention on 8 Trainium2 NeuronCores.

Math (per batch b, head h, with m = hd = 64, scale = hd**-0.25):
  qkv = x @ W_qkv.T ; q,k,v : [N, H, hd]
  phi(z) = exp(scale*z @ rfs[h] - 0.5*|scale*z|^2)          (z = q or k)
  causal scan:  S_t = S_{t-1} + phi_k[t] (x) v[t] ; z_t = z_{t-1} + phi_k[t]
                out[t] = (phi_q[t] @ S_t) / (phi_q[t] . z_t + 1e-16)

Sharding: data-parallel over batch B=8, one batch per core.

Device-side algebra (everything bf16 into the PE, fp32 PSUM accum):
  - rfs[h] is orthogonal * sqrt(hd), so rfs@rfs^T = hd*I. Fold it into the
    projection on the host: proj = x @ (scale * W^T @ rfs), and recover the
    stabilizer |scale*z|^2 = |proj|^2 / hd.
  - The q-side stabilizer exp(-|z_q|^2/2) scales num and den identically and
    cancels in the ratio (den >> 1e-16 for this data), so phi_q = exp(proj_q).
  - k-side keeps it: phi_k = exp(proj_k - |proj_k|^2/(2*hd)), realized by a
    matmul with a constant -1/(2*hd) block-matrix on proj_k^2.

Per-core chunked scan (chunk L=128 tokens, v carries a ones column):
  AT   = phi_k_chunk @ phi_q_chunk^T            [j, i]  (PE, feature-major)
  ATm  = AT * triu_mask                                 (DVE, 4 heads/op)
  num' = phi_q @ [S | z]  +  ATm^T @ [v | 1]    [i, 65] (PE, PSUM-accum)
  S   += phi_k^T @ [v | 1]                              (PE)
  out  = num'[:, :64] / (num'[:, 64] + 1e-16)           (DVE recip, scalar mul)

Layout rule learned on HW: bf16 matmuls whose stationary operand alternates
SBUF base partitions (0 vs 64) crash the PE (concurrent row-tiles into one
PSUM bank).  Everything here keeps lhsT/rhs at base partition 0: odd heads'
phi halves are DMA-copied (partition shift) into their own [64, 512] tiles,
the scan state lives as [64, 780] (head-major columns), and the state PSUM
uses two banks at partitions 0-63.  phi_k token-major tiles come from the
DMA XBAR transpose (16-bit SBUF->SBUF), keeping the PE free for matmuls.
"""

import numpy as np

B, N, C, H = 8, 4096, 768, 12
HD = 64
G = H // 2            # head pairs stacked on 128 partitions
NCH = 512             # tokens per outer chunk
NSUB = NCH // 128     # 128-token scan chunks per outer chunk
NBIG = N // NCH
SCALE = HD ** -0.25

_CACHE = {}


def _build_bass():
    import concourse.bass as bass
    import concourse.mybir as mybir
    import concourse.tile as tile
    from concourse import bacc
    from contextlib import ExitStack

    f32 = mybir.dt.float32
    bf16 = mybir.dt.bfloat16
    AF = mybir.ActivationFunctionType

    nc = bacc.Bacc("TRN2", target_bir_lowering=False)
    xT = nc.declare_dram_parameter("xT", [C, N], bf16, isOutput=False)
    WT = nc.declare_dram_parameter("WT", [C, 3 * C], bf16, isOutput=False)
    nhalf = nc.declare_dram_parameter("nhalf", [128, 128], bf16, isOutput=False)
    mask4 = nc.declare_dram_parameter("mask4", [128, 512], f32, isOutput=False)
    outd = nc.declare_dram_parameter("out", [N, C], f32, isOutput=True)

    with tile.TileContext(nc) as tc, ExitStack() as ctx:
        consts = ctx.enter_context(tc.tile_pool(name="consts", bufs=1))
        xt_p = ctx.enter_context(tc.tile_pool(name="xt", bufs=2))
        phi_p = ctx.enter_context(tc.tile_pool(name="phi", bufs=3))
        odd_p = ctx.enter_context(tc.tile_pool(name="odd", bufs=3))
        sq_p = ctx.enter_context(tc.tile_pool(name="sq", bufs=2))
        tm_p = ctx.enter_context(tc.tile_pool(name="tm", bufs=2))
        v_p = ctx.enter_context(tc.tile_pool(name="v", bufs=2))
        atm_p = ctx.enter_context(tc.tile_pool(name="atm", bufs=3))
        st_p = ctx.enter_context(tc.tile_pool(name="st", bufs=2))
        stb_p = ctx.enter_context(tc.tile_pool(name="stb", bufs=2))
        den_p = ctx.enter_context(tc.tile_pool(name="den", bufs=4))
        out_p = ctx.enter_context(tc.tile_pool(name="outp", bufs=5))

        pp_ps = ctx.enter_context(tc.tile_pool(name="pp", bufs=4, space="PSUM"))
        num_ps = ctx.enter_context(tc.tile_pool(name="nm", bufs=2, space="PSUM"))
        at_ps = ctx.enter_context(tc.tile_pool(name="at", bufs=2, space="PSUM"))

        # ---- constants ----
        wt = []
        for ct in range(6):
            t = consts.tile([128, 3 * C], bf16, tag=f"wt{ct}")
            nc.gpsimd.dma_start(out=t[:], in_=WT[ct * 128:(ct + 1) * 128, :])
            wt.append(t)
        nh_sb = consts.tile([128, 128], bf16, tag="nh")
        nc.gpsimd.dma_start(out=nh_sb[:], in_=nhalf[:])
        mk_sb = consts.tile([128, 512], f32, tag="mk")
        nc.gpsimd.dma_start(out=mk_sb[:], in_=mask4[:])

        prev_f32 = None
        prev_bf = None
        pending_stores = []

        def flush_stores():
            while pending_stores:
                dst, src = pending_stores.pop(0)
                nc.sync.dma_start(out=dst, in_=src)

        def load_xt(cb):
            nsl = slice(cb * NCH, (cb + 1) * NCH)
            xt = []
            for ct in range(6):
                t = xt_p.tile([128, NCH], bf16, tag=f"xt{ct}", name="xt")
                nc.gpsimd.dma_start(out=t[:], in_=xT[ct * 128:(ct + 1) * 128, nsl])
                xt.append(t)
            return xt

        xt = load_xt(0)
        for cb in range(NBIG):
            n0 = cb * NCH

            # ---- v projection (token-major, ones column interleaved) ----
            vsb = []
            for nt in range(NSUB):
                vt = v_p.tile([128, H * 65], bf16, tag=f"v{nt}")
                ones = vt[:].rearrange("p (h d) -> p h d", d=65)[:, :, 64:65]
                nc.vector.memset(ones, 1.0)
                for half in range(2):
                    # v psum rides the num pool: its banks are idle during
                    # the proj phase, and this frees pp slots so k-tile
                    # projections never wait on trailing exps
                    pv = num_ps.tile([128, 512], f32, tag="num", name="pv")
                    fsl = slice(2 * C + half * 384, 2 * C + (half + 1) * 384)
                    for ct in range(6):
                        nc.tensor.matmul(
                            pv[:, :384], xt[ct][:, nt * 128:(nt + 1) * 128],
                            wt[ct][:, fsl],
                            start=(ct == 0), stop=(ct == 5))
                    dst = vt[:, half * 390:(half + 1) * 390].rearrange(
                        "p (h d) -> p h d", d=65)[:, :, :64]
                    src = pv[:, :384].rearrange("p (h d) -> p h d", d=64)
                    nc.scalar.copy(dst, src)
                vsb.append(vt)

            # ---- q/k projection + feature maps (feature-major) ----
            # phi pair tiles hold heads (2g, 2g+1) on partitions 0-63/64-127;
            # odd heads get a DMA partition-shift into base-0 [64, 512] tiles.
            # k's stabilizer chain (sqr -> nhalf-mm -> exp) is deferred one
            # tile so the PE never waits on the scalar square.
            phiq, phik = [None] * H, [None] * H
            tmk = [[None] * NSUB for _ in range(G)]
            pair_k = [None] * G
            pending = []

            def finish_phi(isq, g, ph_):
                dst = phiq if isq == 0 else phik
                dst[2 * g] = ph_
                po = odd_p.tile([64, NCH], bf16, tag=f"po{isq}_{g}")
                nc.gpsimd.dma_start(out=po[:], in_=ph_[64:128, :])
                dst[2 * g + 1] = po
                if isq == 1:
                    pair_k[g] = ph_
                    # transposes ride sync+scalar rings EXCLUSIVELY (mixing
                    # DmaTransposeAnt with regular DMAs on one ring caused
                    # nondeterministic stale reads).  sync's (subs 0,2) go
                    # now; scalar's (subs 1,3) are deferred past the proj
                    # loop so they never delay the sqr/exp chain.
                    for sub in (0, 2):
                        t = tm_p.tile([128, 128], bf16, tag=f"tm{g}_{sub}",
                                      name="tm")
                        nc.sync.dma_start(
                            out=t[:], in_=ph_[:, sub * 128:(sub + 1) * 128],
                            transpose=True)
                        tmk[g][sub] = t

            def flush_pending():
                if not pending:
                    return
                pf_, sqr_, ph_, g_ = pending.pop(0)
                nc.tensor.matmul(pf_[:], nh_sb[:], sqr_[:],
                                 start=False, stop=True, skip_group_check=True)
                nc.scalar.activation(ph_[:], pf_[:], AF.Exp)
                finish_phi(1, g_, ph_)

            for isq in (1, 0):  # k tiles first: tmk transposes start early
                for g in range(G):
                    ft = isq * 6 + g
                    pf = pp_ps.tile([128, NCH], f32, tag="pp", name="pf")
                    for ct in range(6):
                        nc.tensor.matmul(
                            pf[:], wt[ct][:, ft * 128:(ft + 1) * 128], xt[ct][:],
                            start=(ct == 0), stop=(ct == 5))
                    ph = phi_p.tile([128, NCH], bf16, tag=f"ph{isq}_{g}")
                    if isq == 0:
                        flush_pending()
                        nc.scalar.activation(ph[:], pf[:], AF.Exp)
                        finish_phi(0, g, ph)
                    else:
                        flush_pending()
                        sqr = sq_p.tile([128, NCH], bf16, tag="sqr")
                        nc.scalar.square(sqr[:], pf[:])
                        pending.append((pf, sqr, ph, g))
            while pending:
                flush_pending()

            # deferred scalar-ring transposes (subs 1, 3)
            for sub in (1, 3):
                for g in range(G):
                    t = tm_p.tile([128, 128], bf16, tag=f"tm{g}_{sub}",
                                  name="tm")
                    nc.scalar.dma_start(
                        out=t[:], in_=pair_k[g][:, sub * 128:(sub + 1) * 128],
                        transpose=True)
                    tmk[g][sub] = t

            # prefetch next chunk's x^T during this chunk's scan
            xt_next = load_xt(cb + 1) if cb + 1 < NBIG else None

            # previous chunk's stores: normalize long done, so these issue
            # instantly and can't delay the odd-head copies behind them
            flush_stores()

            def phs(lst, h, csl):
                """phi slice for head h over column slice csl, base part. 0."""
                t = lst[h]
                if h % 2 == 0:
                    return t[0:64, csl]
                return t[:, csl]

            # ---- causal scan over 128-token chunks ----
            for sub in range(NSUB):
                ci = cb * NSUB + sub
                ssl = slice(sub * 128, (sub + 1) * 128)
                first = (ci == 0)

                atms = [None] * 3

                def at_group(grp3):
                    pa = at_ps.tile([128, 512], f32, tag="at", name="pa")
                    for k_ in range(4):
                        h = grp3 * 4 + k_
                        nc.tensor.matmul(
                            pa[:, k_ * 128:(k_ + 1) * 128],
                            phs(phik, h, ssl), phs(phiq, h, ssl),
                            start=(k_ == 0), stop=(k_ == 3))
                    atm = atm_p.tile([128, 512], bf16, tag="atm", name="atm")
                    nc.vector.tensor_mul(atm[:], pa[:], mk_sb[:])
                    atms[grp3] = atm

                nums = [num_ps.tile([128, 512], f32, tag="num", name="pn")
                        for _ in range(2)]

                def num_bank(grp6):
                    pn = nums[grp6]
                    for hh in range(6):
                        h = grp6 * 6 + hh
                        opener = (hh == 0)
                        if not first:
                            nc.tensor.matmul(
                                pn[:, hh * 65:hh * 65 + 65],
                                phs(phiq, h, ssl),
                                prev_bf[:, h * 65:(h + 1) * 65],
                                start=opener, stop=False)
                            opener = False
                        nc.tensor.matmul(
                            pn[:, hh * 65:hh * 65 + 65],
                            atms[h // 4][:, (h % 4) * 128:(h % 4 + 1) * 128],
                            vsb[sub][:, h * 65:(h + 1) * 65],
                            start=opener, stop=(hh == 5))

                at_group(0)
                at_group(1)

                # state update: S += phi_k^T @ [v | 1], two banks at part. 0-63
                sts = [pp_ps.tile([128, 512], f32, tag="pp", name="pst")
                       for _ in range(2)]
                for h in range(H):
                    g, e = h // 2, h % 2
                    hh = h % 6
                    nc.tensor.matmul(
                        sts[h // 6][0:64, hh * 65:(hh + 1) * 65],
                        tmk[g][sub][:, e * 64:(e + 1) * 64],
                        vsb[sub][:, h * 65:(h + 1) * 65],
                        start=(hh == 0), stop=(hh == 5))

                num_bank(0)
                at_group(2)
                num_bank(1)

                new_f32 = st_p.tile([64, H * 65], f32, tag="stf")
                if first:
                    nc.vector.tensor_copy(new_f32[:, :390], sts[0][0:64, :390])
                    nc.vector.tensor_copy(new_f32[:, 390:], sts[1][0:64, :390])
                else:
                    nc.vector.tensor_add(new_f32[:, :390], sts[0][0:64, :390],
                                         prev_f32[:, :390])
                    nc.vector.tensor_add(new_f32[:, 390:], sts[1][0:64, :390],
                                         prev_f32[:, 390:])
                new_bf = stb_p.tile([64, H * 65], bf16, tag="stb")
                nc.vector.tensor_copy(new_bf[:], new_f32[:])
                prev_f32, prev_bf = new_f32, new_bf

                # ---- normalize and store ----
                den = den_p.tile([128, H], f32, tag="den")
                for grp6 in range(2):
                    src = nums[grp6][:, :390].rearrange(
                        "p (h d) -> p h d", d=65)[:, :, 64:65]
                    dst = den[:, grp6 * 6:(grp6 + 1) * 6].rearrange(
                        "p (h o) -> p h o", o=1)
                    nc.vector.tensor_copy(dst, src)
                rec = den_p.tile([128, H], f32, tag="rec")
                nc.vector.tensor_scalar_add(rec[:], den[:], 1e-16)
                nc.vector.reciprocal(rec[:], rec[:])
                ot = out_p.tile([128, C], f32, tag="out")
                for h in range(H):
                    nc.vector.tensor_scalar_mul(
                        ot[:, h * 64:(h + 1) * 64],
                        nums[h // 6][:, (h % 6) * 65:(h % 6) * 65 + 64],
                        rec[:, h:h + 1])
                pending_stores.append(
                    (outd[n0 + sub * 128:n0 + (sub + 1) * 128, :], ot[:]))

            xt = xt_next
        flush_stores()

    if not nc.is_finalized():
        nc.finalize()
    return nc


def _host_inputs(x, W_qkv, rfs):
    import ml_dtypes
    bf = ml_dtypes.bfloat16

    x = np.asarray(x, dtype=np.float32)
    W = np.asarray(W_qkv, dtype=np.float64)
    rfs = np.asarray(rfs, dtype=np.float64)

    Wq = W[:C].reshape(H, HD, C)
    Wk = W[C:2 * C].reshape(H, HD, C)
    Wv = W[2 * C:]
    cols = [SCALE * Wq[h].T @ rfs[h] for h in range(H)]
    cols += [SCALE * Wk[h].T @ rfs[h] for h in range(H)]
    cols.append(Wv.T)
    WT = np.ascontiguousarray(np.concatenate(cols, axis=1)).astype(bf)

    nhalf = np.zeros((128, 128), np.float32)
    nhalf[:64, :64] = -1.0 / (2 * HD)
    nhalf[64:, 64:] = -1.0 / (2 * HD)
    nhalf = nhalf.astype(bf)
    mask4 = np.ascontiguousarray(
        np.tile(np.triu(np.ones((128, 128), np.float32)), (1, 4)))

    shared = {"WT": WT, "nhalf": nhalf, "mask4": mask4}
    in_maps = []
    for b in range(B):
        m = {"xT": np.ascontiguousarray(x[b].T).astype(bf)}
        m.update(shared)
        in_maps.append(m)
    return in_maps


def kernel(x, W_qkv, rfs):
    from concourse.bass_utils import run_bass_kernel_spmd

    if "nc" not in _CACHE:
        _CACHE["nc"] = _build_bass()
    nc = _CACHE["nc"]
    in_maps = _host_inputs(x, W_qkv, rfs)
    res = run_bass_kernel_spmd(nc, in_maps, list(range(B)))
    return np.stack([res.results[b]["out"] for b in range(B)], axis=0)


# revision 79
# speedup vs baseline: 1.1436x; 1.0033x over previous
"""FAVOR+ (Performer) linear attention on 8 Trainium2 NeuronCores.

Math (per batch b, head h, with m = hd = 64, scale = hd**-0.25):
  qkv = x @ W_qkv.T ; q,k,v : [N, H, hd]
  phi(z) = exp(scale*z @ rfs[h] - 0.5*|scale*z|^2)          (z = q or k)
  causal scan:  S_t = S_{t-1} + phi_k[t] (x) v[t] ; z_t = z_{t-1} + phi_k[t]
                out[t] = (phi_q[t] @ S_t) / (phi_q[t] . z_t + 1e-16)

Sharding: data-parallel over batch B=8, one batch per core.

Device-side algebra (everything bf16 into the PE, fp32 PSUM accum):
  - rfs[h] is orthogonal * sqrt(hd), so rfs@rfs^T = hd*I. Fold it into the
    projection on the host: proj = x @ (scale * W^T @ rfs), and recover the
    stabilizer |scale*z|^2 = |proj|^2 / hd.
  - The q-side stabilizer exp(-|z_q|^2/2) scales num and den identically and
    cancels in the ratio (den >> 1e-16 for this data), so phi_q = exp(proj_q).
  - k-side keeps it: phi_k = exp(proj_k - |proj_k|^2/(2*hd)), realized by a
    matmul with a constant -1/(2*hd) block-matrix on proj_k^2.

Per-core chunked scan (chunk L=128 tokens, v carries a ones column):
  AT   = phi_k_chunk @ phi_q_chunk^T            [j, i]  (PE, feature-major)
  ATm  = AT * triu_mask                                 (DVE, 4 heads/op)
  num' = phi_q @ [S | z]  +  ATm^T @ [v | 1]    [i, 65] (PE, PSUM-accum)
  S   += phi_k^T @ [v | 1]                              (PE)
  out  = num'[:, :64] / (num'[:, 64] + 1e-16)           (DVE recip, scalar mul)

Layout rule learned on HW: bf16 matmuls whose stationary operand alternates
SBUF base partitions (0 vs 64) crash the PE (concurrent row-tiles into one
PSUM bank).  Everything here keeps lhsT/rhs at base partition 0: odd heads'
phi halves are DMA-copied (partition shift) into their own [64, 512] tiles,
the scan state lives as [64, 780] (head-major columns), and the state PSUM
uses two banks at partitions 0-63.  phi_k token-major tiles come from the
DMA XBAR transpose (16-bit SBUF->SBUF), keeping the PE free for matmuls.
"""

import numpy as np

B, N, C, H = 8, 4096, 768, 12
HD = 64
G = H // 2            # head pairs stacked on 128 partitions
NCH = 512             # tokens per outer chunk
NSUB = NCH // 128     # 128-token scan chunks per outer chunk
NBIG = N // NCH
SCALE = HD ** -0.25

_CACHE = {}


def _build_bass():
    import concourse.bass as bass
    import concourse.mybir as mybir
    import concourse.tile as tile
    from concourse import bacc
    from contextlib import ExitStack

    f32 = mybir.dt.float32
    bf16 = mybir.dt.bfloat16
    AF = mybir.ActivationFunctionType

    nc = bacc.Bacc("TRN2", target_bir_lowering=False)
    xT = nc.declare_dram_parameter("xT", [C, N], bf16, isOutput=False)
    WT = nc.declare_dram_parameter("WT", [C, 3 * C], bf16, isOutput=False)
    nhalf = nc.declare_dram_parameter("nhalf", [128, 128], bf16, isOutput=False)
    mask4 = nc.declare_dram_parameter("mask4", [128, 512], f32, isOutput=False)
    outd = nc.declare_dram_parameter("out", [N, C], f32, isOutput=True)

    with tile.TileContext(nc) as tc, ExitStack() as ctx:
        consts = ctx.enter_context(tc.tile_pool(name="consts", bufs=1))
        xt_p = ctx.enter_context(tc.tile_pool(name="xt", bufs=2))
        phi_p = ctx.enter_context(tc.tile_pool(name="phi", bufs=3))
        odd_p = ctx.enter_context(tc.tile_pool(name="odd", bufs=3))
        sq_p = ctx.enter_context(tc.tile_pool(name="sq", bufs=2))
        tm_p = ctx.enter_context(tc.tile_pool(name="tm", bufs=3))
        v_p = ctx.enter_context(tc.tile_pool(name="v", bufs=3))
        atm_p = ctx.enter_context(tc.tile_pool(name="atm", bufs=3))
        st_p = ctx.enter_context(tc.tile_pool(name="st", bufs=2))
        stb_p = ctx.enter_context(tc.tile_pool(name="stb", bufs=2))
        den_p = ctx.enter_context(tc.tile_pool(name="den", bufs=4))
        out_p = ctx.enter_context(tc.tile_pool(name="outp", bufs=5))

        pp_ps = ctx.enter_context(tc.tile_pool(name="pp", bufs=4, space="PSUM"))
        num_ps = ctx.enter_context(tc.tile_pool(name="nm", bufs=2, space="PSUM"))
        at_ps = ctx.enter_context(tc.tile_pool(name="at", bufs=2, space="PSUM"))

        # ---- constants ----
        wt = []
        for ct in range(6):
            t = consts.tile([128, 3 * C], bf16, tag=f"wt{ct}")
            nc.gpsimd.dma_start(out=t[:], in_=WT[ct * 128:(ct + 1) * 128, :])
            wt.append(t)
        nh_sb = consts.tile([128, 128], bf16, tag="nh")
        nc.gpsimd.dma_start(out=nh_sb[:], in_=nhalf[:])
        mk_sb = consts.tile([128, 512], f32, tag="mk")
        nc.gpsimd.dma_start(out=mk_sb[:], in_=mask4[:])

        prev_f32 = None
        prev_bf = None
        pending_stores = []

        def flush_stores():
            while pending_stores:
                dst, src = pending_stores.pop(0)
                nc.sync.dma_start(out=dst, in_=src)

        def load_xt(cb):
            nsl = slice(cb * NCH, (cb + 1) * NCH)
            xt = []
            for ct in range(6):
                t = xt_p.tile([128, NCH], bf16, tag=f"xt{ct}", name="xt")
                nc.gpsimd.dma_start(out=t[:], in_=xT[ct * 128:(ct + 1) * 128, nsl])
                xt.append(t)
            return xt

        xt = load_xt(0)
        for cb in range(NBIG):
            n0 = cb * NCH

            # ---- v projection (token-major, ones column interleaved) ----
            vsb = []
            for nt in range(NSUB):
                vt = v_p.tile([128, H * 65], bf16, tag=f"v{nt}")
                ones = vt[:].rearrange("p (h d) -> p h d", d=65)[:, :, 64:65]
                nc.vector.memset(ones, 1.0)
                for half in range(2):
                    # v psum rides the num pool: its banks are idle during
                    # the proj phase, and this frees pp slots so k-tile
                    # projections never wait on trailing exps
                    pv = num_ps.tile([128, 512], f32, tag="num", name="pv")
                    fsl = slice(2 * C + half * 384, 2 * C + (half + 1) * 384)
                    for ct in range(6):
                        nc.tensor.matmul(
                            pv[:, :384], xt[ct][:, nt * 128:(nt + 1) * 128],
                            wt[ct][:, fsl],
                            start=(ct == 0), stop=(ct == 5))
                    dst = vt[:, half * 390:(half + 1) * 390].rearrange(
                        "p (h d) -> p h d", d=65)[:, :, :64]
                    src = pv[:, :384].rearrange("p (h d) -> p h d", d=64)
                    nc.scalar.copy(dst, src)
                vsb.append(vt)

            # ---- q/k projection + feature maps (feature-major) ----
            # phi pair tiles hold heads (2g, 2g+1) on partitions 0-63/64-127;
            # odd heads get a DMA partition-shift into base-0 [64, 512] tiles.
            # k's stabilizer chain (sqr -> nhalf-mm -> exp) is deferred one
            # tile so the PE never waits on the scalar square.
            phiq, phik = [None] * H, [None] * H
            tmk = [[None] * NSUB for _ in range(G)]
            pair_k = [None] * G
            pending = []

            def finish_phi(isq, g, ph_):
                dst = phiq if isq == 0 else phik
                dst[2 * g] = ph_
                po = odd_p.tile([64, NCH], bf16, tag=f"po{isq}_{g}")
                nc.gpsimd.dma_start(out=po[:], in_=ph_[64:128, :])
                dst[2 * g + 1] = po
                if isq == 1:
                    pair_k[g] = ph_
                    # transposes ride sync+scalar rings EXCLUSIVELY (mixing
                    # DmaTransposeAnt with regular DMAs on one ring caused
                    # nondeterministic stale reads).  sync's (subs 0,2) go
                    # now; scalar's (subs 1,3) are deferred past the proj
                    # loop so they never delay the sqr/exp chain.
                    for sub in (0, 2):
                        t = tm_p.tile([128, 128], bf16, tag=f"tm{g}_{sub}",
                                      name="tm")
                        nc.sync.dma_start(
                            out=t[:], in_=ph_[:, sub * 128:(sub + 1) * 128],
                            transpose=True)
                        tmk[g][sub] = t

            def flush_pending():
                if not pending:
                    return
                pf_, sqr_, ph_, g_ = pending.pop(0)
                nc.tensor.matmul(pf_[:], nh_sb[:], sqr_[:],
                                 start=False, stop=True, skip_group_check=True)
                nc.scalar.activation(ph_[:], pf_[:], AF.Exp)
                finish_phi(1, g_, ph_)

            for isq in (1, 0):  # k tiles first: tmk transposes start early
                for g in range(G):
                    ft = isq * 6 + g
                    pf = pp_ps.tile([128, NCH], f32, tag="pp", name="pf")
                    for ct in range(6):
                        nc.tensor.matmul(
                            pf[:], wt[ct][:, ft * 128:(ft + 1) * 128], xt[ct][:],
                            start=(ct == 0), stop=(ct == 5))
                    ph = phi_p.tile([128, NCH], bf16, tag=f"ph{isq}_{g}")
                    if isq == 0:
                        flush_pending()
                        nc.scalar.activation(ph[:], pf[:], AF.Exp)
                        finish_phi(0, g, ph)
                    else:
                        flush_pending()
                        sqr = sq_p.tile([128, NCH], bf16, tag="sqr")
                        nc.scalar.square(sqr[:], pf[:])
                        pending.append((pf, sqr, ph, g))
            while pending:
                flush_pending()

            # deferred scalar-ring transposes (subs 1, 3)
            for sub in (1, 3):
                for g in range(G):
                    t = tm_p.tile([128, 128], bf16, tag=f"tm{g}_{sub}",
                                  name="tm")
                    nc.scalar.dma_start(
                        out=t[:], in_=pair_k[g][:, sub * 128:(sub + 1) * 128],
                        transpose=True)
                    tmk[g][sub] = t

            # prefetch next chunk's x^T during this chunk's scan
            xt_next = load_xt(cb + 1) if cb + 1 < NBIG else None

            # previous chunk's stores: normalize long done, so these issue
            # instantly and can't delay the odd-head copies behind them
            flush_stores()

            def phs(lst, h, csl):
                """phi slice for head h over column slice csl, base part. 0."""
                t = lst[h]
                if h % 2 == 0:
                    return t[0:64, csl]
                return t[:, csl]

            # ---- causal scan over 128-token chunks ----
            for sub in range(NSUB):
                ci = cb * NSUB + sub
                ssl = slice(sub * 128, (sub + 1) * 128)
                first = (ci == 0)

                atms = [None] * 3

                def at_group(grp3):
                    pa = at_ps.tile([128, 512], f32, tag="at", name="pa")
                    for k_ in range(4):
                        h = grp3 * 4 + k_
                        nc.tensor.matmul(
                            pa[:, k_ * 128:(k_ + 1) * 128],
                            phs(phik, h, ssl), phs(phiq, h, ssl),
                            start=(k_ == 0), stop=(k_ == 3))
                    atm = atm_p.tile([128, 512], bf16, tag="atm", name="atm")
                    nc.vector.tensor_mul(atm[:], pa[:], mk_sb[:])
                    atms[grp3] = atm

                nums = [num_ps.tile([128, 512], f32, tag="num", name="pn")
                        for _ in range(2)]

                def num_bank(grp6):
                    pn = nums[grp6]
                    for hh in range(6):
                        h = grp6 * 6 + hh
                        opener = (hh == 0)
                        if not first:
                            nc.tensor.matmul(
                                pn[:, hh * 65:hh * 65 + 65],
                                phs(phiq, h, ssl),
                                prev_bf[:, h * 65:(h + 1) * 65],
                                start=opener, stop=False)
                            opener = False
                        nc.tensor.matmul(
                            pn[:, hh * 65:hh * 65 + 65],
                            atms[h // 4][:, (h % 4) * 128:(h % 4 + 1) * 128],
                            vsb[sub][:, h * 65:(h + 1) * 65],
                            start=opener, stop=(hh == 5))

                at_group(0)
                at_group(1)

                # state update: S += phi_k^T @ [v | 1], two banks at part. 0-63
                sts = [pp_ps.tile([128, 512], f32, tag="pp", name="pst")
                       for _ in range(2)]
                for h in range(H):
                    g, e = h // 2, h % 2
                    hh = h % 6
                    nc.tensor.matmul(
                        sts[h // 6][0:64, hh * 65:(hh + 1) * 65],
                        tmk[g][sub][:, e * 64:(e + 1) * 64],
                        vsb[sub][:, h * 65:(h + 1) * 65],
                        start=(hh == 0), stop=(hh == 5))

                num_bank(0)
                at_group(2)
                num_bank(1)

                new_f32 = st_p.tile([64, H * 65], f32, tag="stf")
                if first:
                    nc.vector.tensor_copy(new_f32[:, :390], sts[0][0:64, :390])
                    nc.vector.tensor_copy(new_f32[:, 390:], sts[1][0:64, :390])
                else:
                    nc.vector.tensor_add(new_f32[:, :390], sts[0][0:64, :390],
                                         prev_f32[:, :390])
                    nc.vector.tensor_add(new_f32[:, 390:], sts[1][0:64, :390],
                                         prev_f32[:, 390:])
                new_bf = stb_p.tile([64, H * 65], bf16, tag="stb")
                nc.vector.tensor_copy(new_bf[:], new_f32[:])
                prev_f32, prev_bf = new_f32, new_bf

                # ---- normalize and store ----
                den = den_p.tile([128, H], f32, tag="den")
                for grp6 in range(2):
                    src = nums[grp6][:, :390].rearrange(
                        "p (h d) -> p h d", d=65)[:, :, 64:65]
                    dst = den[:, grp6 * 6:(grp6 + 1) * 6].rearrange(
                        "p (h o) -> p h o", o=1)
                    nc.vector.tensor_copy(dst, src)
                rec = den_p.tile([128, H], f32, tag="rec")
                nc.vector.tensor_scalar_add(rec[:], den[:], 1e-16)
                nc.vector.reciprocal(rec[:], rec[:])
                ot = out_p.tile([128, C], f32, tag="out")
                for h in range(H):
                    nc.vector.tensor_scalar_mul(
                        ot[:, h * 64:(h + 1) * 64],
                        nums[h // 6][:, (h % 6) * 65:(h % 6) * 65 + 64],
                        rec[:, h:h + 1])
                pending_stores.append(
                    (outd[n0 + sub * 128:n0 + (sub + 1) * 128, :], ot[:]))

            xt = xt_next
        flush_stores()

    if not nc.is_finalized():
        nc.finalize()
    return nc


def _host_inputs(x, W_qkv, rfs):
    import ml_dtypes
    bf = ml_dtypes.bfloat16

    x = np.asarray(x, dtype=np.float32)
    W = np.asarray(W_qkv, dtype=np.float64)
    rfs = np.asarray(rfs, dtype=np.float64)

    Wq = W[:C].reshape(H, HD, C)
    Wk = W[C:2 * C].reshape(H, HD, C)
    Wv = W[2 * C:]
    cols = [SCALE * Wq[h].T @ rfs[h] for h in range(H)]
    cols += [SCALE * Wk[h].T @ rfs[h] for h in range(H)]
    cols.append(Wv.T)
    WT = np.ascontiguousarray(np.concatenate(cols, axis=1)).astype(bf)

    nhalf = np.zeros((128, 128), np.float32)
    nhalf[:64, :64] = -1.0 / (2 * HD)
    nhalf[64:, 64:] = -1.0 / (2 * HD)
    nhalf = nhalf.astype(bf)
    mask4 = np.ascontiguousarray(
        np.tile(np.triu(np.ones((128, 128), np.float32)), (1, 4)))

    shared = {"WT": WT, "nhalf": nhalf, "mask4": mask4}
    in_maps = []
    for b in range(B):
        m = {"xT": np.ascontiguousarray(x[b].T).astype(bf)}
        m.update(shared)
        in_maps.append(m)
    return in_maps


def kernel(x, W_qkv, rfs):
    from concourse.bass_utils import run_bass_kernel_spmd

    if "nc" not in _CACHE:
        _CACHE["nc"] = _build_bass()
    nc = _CACHE["nc"]
    in_maps = _host_inputs(x, W_qkv, rfs)
    res = run_bass_kernel_spmd(nc, in_maps, list(range(B)))
    return np.stack([res.results[b]["out"] for b in range(B)], axis=0)
